# revision 9
# baseline (speedup 1.0000x reference)
"""AELN-GCN (edge-weighted ChebConv K=3, 4 layers) on 8 TRN2 NeuronCores.

v2: gather-descriptor-bound redesign.

Profiling v1 showed the run was bound by SWDGE descriptor generation for
dma_gather on the GpSimd engine (~8.8ns/row, 82.9% busy).  The ucode runs
each dma_gather on one Q7 core-pair selected by queue_num, so v2:
  - compiles with num_swdge_queues=4 and stripes gather calls across the
    4 queues (4 core-pairs emit descriptors concurrently, ~2.7x).
  - buckets the all-gathered table by shard-quarter so each bucket is
    25088 rows (int16-indexable) and issues one gather call per
    (dst tile, bucket) on queue=bucket.
  - precomputes the one-hot aggregation matrices Aw = onehot(dstrel) * ew
    into DRAM once (they are layer-invariant) and streams them per prop,
    removing the per-chunk is_equal/scale from the propagate loop.
  - keeps x_temp/t1/t2/drep resident in bf16 so cheb matmuls read them
    directly.
  - splits each table AllGather into 4 quarter collectives so the next
    prop's gathers start as soon as their bucket arrives.

Math per propagate (unchanged): y = -dinv[dst] * sum(ew * dinv[src] * x[src])
with the dinv[src] folded into the table rows and -dinv[dst] applied at
PSUM evacuation via a replicated drep tile.
"""

import sys

sys.path.insert(0, "/opt/trn_rl_repo")

import numpy as np
import ml_dtypes

P = 128  # partitions
EF = 64  # edge feature dim
HF = 32  # edgenet hidden dim
F = 128  # node feature dim
NGL = 4
NQ = 4  # shard quarters = gather buckets = swdge queues
GCAP = 1024  # HW cap on idxs per dma_gather call

BF16 = ml_dtypes.bfloat16


# ----------------------------------------------------------------------------
# config
# ----------------------------------------------------------------------------
class Cfg:
    def __init__(self, n_nodes, n_edges, n_cores):
        assert n_nodes % n_cores == 0
        self.N = n_nodes
        self.E = n_edges
        self.NC = n_cores
        self.NSH = n_nodes // n_cores  # real nodes per shard
        self.DT = -(-self.NSH // P)  # dst tiles per core
        self.DTP = self.DT * P  # padded shard rows
        assert self.DTP % NQ == 0
        self.QS = self.DTP // NQ  # quarter size (rows per shard-quarter)
        self.BSZ = self.NC * self.QS  # bucket rows (per-quarter table)
        assert self.BSZ <= 32768
        self.TROWS = self.NC * self.DTP
        # set by host_prep (uniform across cores):
        self.CTQ = None  # [DT, NQ] chunks per (tile, bucket)
        self.CT = None  # total chunks (sum of CTQ)
        self.CT2 = None  # chunks per src tile (deg phase)
        self.NIDX = None  # [DT, NQ] static num_idxs per gather call
        self.CTB = 0  # legacy (cache key)

    @property
    def S(self):
        return self.CT * P  # uniform slot count

    @property
    def EP2(self):
        return self.DT * self.CT2 * P


# ----------------------------------------------------------------------------
# host prep
# ----------------------------------------------------------------------------
def _shard_order(node_of_edge, cfg):
    """Per-core edge lists grouped by local tile of `node_of_edge`."""
    cores = []
    max_ct = 1
    order = np.argsort(node_of_edge, kind="stable")
    node_sorted = node_of_edge[order]
    for c in range(cfg.NC):
        lo = np.searchsorted(node_sorted, c * cfg.NSH)
        hi = np.searchsorted(node_sorted, (c + 1) * cfg.NSH)
        eids = order[lo:hi]
        locs = node_sorted[lo:hi] - c * cfg.NSH
        tiles = locs // P
        counts = np.bincount(tiles, minlength=cfg.DT)
        max_ct = max(max_ct, int(-(-counts.max() // P)))
        cores.append((eids, locs, tiles, counts))
    return cores, max_ct


def _fill_slots(cores, cfg, ct):
    """Baseline-style [DT, P, CT] layout (for the deg phase)."""
    out = []
    for eids, locs, tiles, counts in cores:
        eid = np.full((cfg.DT, ct * P), -1, dtype=np.int64)
        rel = np.full((cfg.DT, ct * P), -1, dtype=np.int32)
        starts = np.zeros(cfg.DT + 1, dtype=np.int64)
        np.cumsum(counts, out=starts[1:])
        for t in range(cfg.DT):
            n_t = counts[t]
            if n_t == 0:
                continue
            sl = slice(starts[t], starts[t + 1])
            eid[t, :n_t] = eids[sl]
            rel[t, :n_t] = locs[sl] - t * P
        eid = eid.reshape(cfg.DT, ct, P).transpose(0, 2, 1).copy()
        rel = rel.reshape(cfg.DT, ct, P).transpose(0, 2, 1).copy()
        out.append((eid, rel))
    return out


def host_prep(inputs, cfg):
    feats = np.asarray(inputs["features"], dtype=np.float32)
    egin = np.asarray(inputs["edgenet_input"], dtype=np.float32)
    E1 = np.asarray(inputs["E1"], dtype=np.float32)
    e1b = np.asarray(inputs["e1b"], dtype=np.float32)
    E2 = np.asarray(inputs["E2"], dtype=np.float32)
    e2b = np.asarray(inputs["e2b"], dtype=np.float32)
    W = np.asarray(inputs["W"], dtype=np.float32)
    b = np.asarray(inputs["b"], dtype=np.float32)
    ei = np.asarray(inputs["edge_index"])
    src = ei[0].astype(np.int64)
    dst = ei[1].astype(np.int64)

    # node -> (bucket q, row within bucket)
    shard = src // cfg.NSH
    local = src % cfg.NSH  # < NSH <= DTP
    q_of_src = local // cfg.QS
    brow_of_src = shard * cfg.QS + (local % cfg.QS)

    # ---- dst-shard slot assignment, grouped by (tile, src-bucket) ----
    dcores, _ = _shard_order(dst, cfg)
    percore = []
    cnt_tq = np.zeros((cfg.NC, cfg.DT, NQ), dtype=np.int64)
    for c, (eids, locs, tiles, counts) in enumerate(dcores):
        q = q_of_src[eids]
        key = tiles * NQ + q
        order = np.argsort(key * (cfg.BSZ + 1) + brow_of_src[eids], kind="stable")
        eids_s = eids[order]
        key_s = key[order]
        cnt = np.bincount(key_s, minlength=cfg.DT * NQ).reshape(cfg.DT, NQ)
        cnt_tq[c] = cnt
        percore.append((eids_s, locs[order], key_s))

    # uniform chunk layout from max-over-cores counts
    maxcnt = cnt_tq.max(axis=0)  # [DT, NQ]
    # round num_idxs up to full 128-slot chunks: every slot is written by the
    # gather (pad idxs hit row 0), so no uninitialized SBUF reaches the matmul
    # (0 * garbage-NaN would poison PSUM).
    nidx = -(-np.maximum(maxcnt, 128) // P) * P  # static num_idxs, %128
    ctq = nidx // P  # chunks per (t, q)
    # split oversize calls is not supported; assert under HW cap
    assert nidx.max() <= GCAP, nidx.max()
    cfg.CTQ = ctq
    cfg.NIDX = nidx
    ct_t = ctq.sum(axis=1)  # [DT]
    cfg.CT = int(ct_t.sum())
    chunk_base = np.zeros(cfg.DT, dtype=np.int64)  # first chunk id of tile
    np.cumsum(ct_t[:-1], out=chunk_base[1:])

    # src-shard layout for deg (unchanged from v1)
    scores, ct_s = _shard_order(src, cfg)
    cfg.CT2 = ct_s
    while (cfg.DT * cfg.CT2) % 4 != 0:
        cfg.CT2 += 1
    sslots = _fill_slots(scores, cfg, cfg.CT2)

    eg_ext = np.concatenate([egin, np.zeros((1, EF), np.float32)], axis=0)

    S = cfg.S
    in_maps = []
    for c in range(cfg.NC):
        eids_s, locs_s, key_s = percore[c]
        # slot id per edge: chunks laid out [tile][q-segments]
        eid_slot = np.full(S, -1, dtype=np.int64)  # edge id per slot
        rel_slot = np.full(S, -1, dtype=np.int32)  # dstrel per slot
        gidx_cols = int(nidx.sum() // 16)
        gidx = np.zeros((16, gidx_cols), dtype=np.int16)
        starts = np.zeros(cfg.DT * NQ + 1, dtype=np.int64)
        np.cumsum(cnt_tq[c].reshape(-1), out=starts[1:])
        gcol = 0
        for t in range(cfg.DT):
            sbase = chunk_base[t] * P
            joff = 0
            for q in range(NQ):
                n_c = int(cnt_tq[c, t, q])
                sl = slice(starts[t * NQ + q], starts[t * NQ + q] + n_c)
                es = eids_s[sl]
                s0 = sbase + joff * P
                eid_slot[s0 : s0 + n_c] = es
                rel_slot[s0 : s0 + n_c] = locs_s[sl] - t * P
                # gather idx payload for this call, padded to nidx[t, q]
                n4 = int(nidx[t, q])
                idxs = np.zeros(n4, dtype=np.int16)
                if n_c > 0:
                    idxs[:n_c] = brow_of_src[es].astype(np.int16)
                ncols = n4 // 16
                gidx[:, gcol : gcol + ncols] = idxs.reshape(ncols, 16).T
                gcol += ncols
                joff += int(ctq[t, q])
        assert gcol == gidx_cols
        gidx_t = np.ascontiguousarray(np.tile(gidx, (8, 1)))

        # per-slot streams in chunk-major [chunk, p] order
        eg = eg_ext[np.where(eid_slot >= 0, eid_slot, len(egin))]  # [S, 64]
        dstrel = rel_slot.reshape(cfg.CT, P).T.astype(BF16)  # [P, CT]

        eid_s2, rel_s2 = sslots[c]

        featT = np.zeros((F, cfg.DTP), BF16)
        sh = feats[c * cfg.NSH : (c + 1) * cfg.NSH]
        featT[:, : cfg.NSH] = sh.T.astype(BF16)

        m = {
            "featT": featT,  # [F, DTP] bf16
            "eg": np.ascontiguousarray(eg.T.astype(BF16)),  # [64, S] bf16
            "eg2": np.ascontiguousarray(
                eg_ext[eid_s2.reshape(-1)].T.astype(BF16)
            ),  # [64, EP2]
            "gidx": gidx_t,  # [128, gidx_cols] i16
            "dstrel": dstrel,  # [P, CT] bf16
            "srcrel2": rel_s2.astype(np.float32),  # [DT, P, CT2] f32
            "E1": E1.astype(BF16),
            "e1b": e1b.reshape(HF, 1),
            "E2": E2.astype(BF16),
            "e2b": e2b.reshape(1, 1),
            "W": W,
            "bT": np.ascontiguousarray(b.T),  # [128, NGL]
        }
        in_maps.append(m)
    return in_maps, None


# ----------------------------------------------------------------------------
# assemble final output
# ----------------------------------------------------------------------------
def assemble(results, cfg):
    out = np.zeros((cfg.N, F), np.float32)
    for c in range(cfg.NC):
        outT = np.asarray(results[c]["outT"], dtype=np.float32)
        out[c * cfg.NSH : (c + 1) * cfg.NSH] = outT[:, : cfg.NSH].T
    return out


# ----------------------------------------------------------------------------
# bass kernel builder
# ----------------------------------------------------------------------------
def build_nc(cfg, do_compile=True):
    import concourse.bass as bass
    import concourse.bacc as bacc
    import concourse.mybir as mybir
    import concourse.tile as tile
    from concourse.masks import make_identity

    dt = mybir.dt
    AF = mybir.ActivationFunctionType
    ALU = mybir.AluOpType

    NC, DT, CT, CT2, DTP = cfg.NC, cfg.DT, cfg.CT, cfg.CT2, cfg.DTP
    QS, BSZ = cfg.QS, cfg.BSZ
    CTQ, NIDX = cfg.CTQ, cfg.NIDX
    S, EP2 = cfg.S, cfg.EP2
    core_ids = list(range(NC))
    ct_t = CTQ.sum(axis=1)
    CTMAX = int(ct_t.max())
    chunk_base = np.zeros(DT, dtype=np.int64)
    np.cumsum(ct_t[:-1], out=chunk_base[1:])
    # gather idx column offset per (t, q)
    gidx_off = np.zeros((DT, NQ), dtype=np.int64)
    np.cumsum((NIDX // 16).reshape(-1)[:-1], out=gidx_off.reshape(-1)[1:])
    GIDX_COLS = int((NIDX // 16).sum())
    # last tile index touching each shard quarter (gates the quarter AllGather)
    ag_tile = [min(DT - 1, (QS * (q + 1) - 1) // P) for q in range(NQ)]

    nc = bacc.Bacc(num_swdge_queues=NQ)

    # ---- external I/O ----
    featT_d = nc.declare_dram_parameter("featT", [F, DTP], dt.bfloat16, isOutput=False)
    eg_d = nc.declare_dram_parameter("eg", [EF, S], dt.bfloat16, isOutput=False)
    eg2_d = nc.declare_dram_parameter("eg2", [EF, EP2], dt.bfloat16, isOutput=False)
    gidx_d = nc.declare_dram_parameter("gidx", [P, GIDX_COLS], dt.int16, isOutput=False)
    dstrel_d = nc.declare_dram_parameter("dstrel", [P, CT], dt.bfloat16, isOutput=False)
    srcrel2_d = nc.declare_dram_parameter(
        "srcrel2", [DT, P, CT2], dt.float32, isOutput=False
    )
    E1_d = nc.declare_dram_parameter("E1", [EF, HF], dt.bfloat16, isOutput=False)
    e1b_d = nc.declare_dram_parameter("e1b", [HF, 1], dt.float32, isOutput=False)
    E2_d = nc.declare_dram_parameter("E2", [HF, 1], dt.bfloat16, isOutput=False)
    e2b_d = nc.declare_dram_parameter("e2b", [1, 1], dt.float32, isOutput=False)
    W_d = nc.declare_dram_parameter("W", [NGL, 3, F, F], dt.float32, isOutput=False)
    bT_d = nc.declare_dram_parameter("bT", [F, NGL], dt.float32, isOutput=False)
    outT_d = nc.declare_dram_parameter("outT", [F, DTP], dt.float32, isOutput=True)

    # ---- internal dram ----
    ew1_d = nc.dram_tensor("ew1_buf", [S], dt.float32)
    ew2_d = nc.dram_tensor("ew2_buf", [EP2], dt.float32)
    dinv_d = nc.dram_tensor("dinv_buf", [DTP], dt.float32)
    aw_d = nc.dram_tensor("aw_buf", [P, CT * P], dt.bfloat16)
    xsrc_a = nc.dram_tensor("xsrc_a", [DTP, F], dt.bfloat16)
    xsrc_b = nc.dram_tensor("xsrc_b", [DTP, F], dt.bfloat16)
    xfull_a = [
        nc.dram_tensor(f"xfull_a{q}", [BSZ, F], dt.bfloat16, addr_space="Shared")
        for q in range(NQ)
    ]
    xfull_b = [
        nc.dram_tensor(f"xfull_b{q}", [BSZ, F], dt.bfloat16, addr_space="Shared")
        for q in range(NQ)
    ]

    EB = 512  # edgenet block columns (psum bank limit)

    with tile.TileContext(nc, num_cores=NC) as tc:
        with (
            tc.tile_pool(name="res", bufs=1) as res,
            tc.tile_pool(name="resw", bufs=1) as resw,
        ):
            # ---- resident tiles ----
            x_temp = res.tile([F, DTP], dt.bfloat16, tag="x_temp")
            t1 = res.tile([F, DTP], dt.bfloat16, tag="t1")
            t2 = res.tile([F, DTP], dt.bfloat16, tag="t2")
            drep1 = res.tile([F, DTP], dt.bfloat16, tag="drep1")  # -dinv replicated
            dinv_sb = res.tile([P, DT], dt.float32, tag="dinv_sb")
            deg_sb = res.tile([P, DT], dt.float32, tag="deg_sb")
            ident_f = res.tile([P, P], dt.float32, tag="ident_f")
            ident_b = res.tile([P, P], dt.bfloat16, tag="ident_b")
            iota_i = res.tile([P, P], dt.int32, tag="iota_i")
            iota_b = res.tile([P, P], dt.bfloat16, tag="iota_b")
            iota_f = res.tile([P, P], dt.float32, tag="iota_f")
            ones_sb = res.tile([1, P], dt.float32, tag="ones_sb")
            gidx_sb = res.tile([P, GIDX_COLS], dt.int16, tag="gidx_sb")
            E1_sb = resw.tile([EF, HF], dt.bfloat16, tag="E1_sb")
            e1b_sb = resw.tile([HF, 1], dt.float32, tag="e1b_sb")
            E2_sb = resw.tile([HF, 1], dt.bfloat16, tag="E2_sb")
            e2b_sb = resw.tile([1, 1], dt.float32, tag="e2b_sb")
            bT_sb = resw.tile([F, NGL], dt.float32, tag="bT_sb")
            W0_sb = resw.tile([F, NGL * F], dt.bfloat16, tag="W0_sb")
            Wb_sb = resw.tile([F, NGL * 2 * F], dt.bfloat16, tag="Wb_sb")

            make_identity(nc, ident_f[:])
            nc.vector.tensor_copy(ident_b[:], ident_f[:])
            nc.gpsimd.iota(iota_i[:], pattern=[[1, P]], base=0, channel_multiplier=0)
            nc.vector.tensor_copy(iota_b[:], iota_i[:])
            nc.vector.tensor_copy(iota_f[:], iota_i[:])
            nc.gpsimd.memset(ones_sb[:], 1.0)
            nc.sync.dma_start(out=gidx_sb[:], in_=gidx_d[:, :])
            nc.sync.dma_start(out=E1_sb[:], in_=E1_d[:, :])
            nc.sync.dma_start(out=e1b_sb[:], in_=e1b_d[:, :])
            nc.sync.dma_start(out=E2_sb[:], in_=E2_d[:, :])
            nc.sync.dma_start(out=e2b_sb[:], in_=e2b_d[:, :])
            nc.sync.dma_start(out=bT_sb[:], in_=bT_d[:, :])
            for l in range(NGL):
                nc.gpsimd.dma_start(
                    out=W0_sb[:, l * F : (l + 1) * F], in_=W_d[l, 0, :, :]
                )
                for k in (1, 2):
                    nc.gpsimd.dma_start(
                        out=Wb_sb[:, (l * 2 + k - 1) * F : (l * 2 + k) * F],
                        in_=W_d[l, k, :, :],
                    )
            nc.sync.dma_start(out=x_temp[:], in_=featT_d[:, :])

            # ================= phase 1: edgenet (both orders) + deg =========
            with (
                tc.tile_pool(name="egp", bufs=3) as egp,
                tc.tile_pool(name="hp", bufs=2) as hp,
                tc.tile_pool(name="hpp", bufs=2, space="PSUM") as hpp,
                tc.tile_pool(name="ewp", bufs=2) as ewp,
                tc.tile_pool(name="ewpp", bufs=2, space="PSUM") as ewpp,
                tc.tile_pool(name="degp", bufs=3) as degp,
                tc.tile_pool(name="degpp", bufs=2, space="PSUM") as degpp,
            ):

                def edgenet(eg_dram, ew_dram, n_cols):
                    c0 = 0
                    while c0 < n_cols:
                        nb = min(EB, n_cols - c0)
                        egt = egp.tile([EF, EB], dt.bfloat16, tag="egt")
                        nc.sync.dma_start(
                            out=egt[:, :nb], in_=eg_dram[:, c0 : c0 + nb]
                        )
                        ph = hpp.tile([HF, EB], dt.float32, tag="ph", space="PSUM")
                        nc.tensor.matmul(
                            out=ph[:, :nb], lhsT=E1_sb[:], rhs=egt[:, :nb], start=True, stop=True
                        )
                        hs = hp.tile([HF, EB], dt.bfloat16, tag="hs")
                        nc.scalar.activation(hs[:, :nb], ph[:, :nb], AF.Relu, bias=e1b_sb[:, :1])
                        pw = ewpp.tile([1, EB], dt.float32, tag="pw", space="PSUM")
                        nc.tensor.matmul(
                            out=pw[:, :nb], lhsT=E2_sb[:], rhs=hs[:, :nb], start=True, stop=True
                        )
                        ews = ewp.tile([1, EB], dt.float32, tag="ews")
                        nc.scalar.activation(
                            ews[:, :nb], pw[:, :nb], AF.Sigmoid, bias=e2b_sb[:1, :1]
                        )
                        nc.sync.dma_start(
                            out=ew_dram[None, c0 : c0 + nb],
                            in_=ews[:1, :nb],
                        )
                        c0 += nb

                edgenet(eg2_d, ew2_d, EP2)

                # deg: per src tile, one-hot matmul accumulate
                for t in range(DT):
                    relt = degp.tile([P, CT2], dt.float32, tag="relt")
                    nc.sync.dma_start(out=relt[:], in_=srcrel2_d[t, :, :])
                    ewt = degp.tile([P, CT2], dt.float32, tag="ewt2")
                    nc.sync.dma_start(
                        out=ewt[:],
                        in_=ew2_d[t * P * CT2 : (t + 1) * P * CT2].rearrange(
                            "(p j) -> p j", p=P
                        ),
                    )
                    pd = degpp.tile([P, 1], dt.float32, tag="pd", space="PSUM", bufs=1)
                    for j in range(CT2):
                        A2 = degp.tile([P, P], dt.float32, tag="A2")
                        nc.vector.tensor_tensor(
                            out=A2[:],
                            in0=relt[:, j : j + 1].to_broadcast([P, P]),
                            in1=iota_f[:],
                            op=ALU.is_equal,
                        )
                        nc.tensor.matmul(
                            out=pd[:],
                            lhsT=A2[:],
                            rhs=ewt[:, j : j + 1],
                            start=(j == 0),
                            stop=(j == CT2 - 1),
                        )
                    nc.vector.tensor_copy(deg_sb[:, t : t + 1], pd[:])

                edgenet(eg_d, ew1_d, S)

                # dinv = (deg>0) / sqrt(max(deg,1e-12))
                mx = degp.tile([P, DT], dt.float32, tag="mx")
                nc.vector.tensor_scalar_max(mx[:], deg_sb[:], 1e-12)
                rc = degp.tile([P, DT], dt.float32, tag="rc")
                nc.vector.reciprocal(rc[:], mx[:])
                sq = degp.tile([P, DT], dt.float32, tag="sq")
                nc.scalar.activation(sq[:], rc[:], AF.Sqrt)
                msk = degp.tile([P, DT], dt.float32, tag="msk")
                nc.vector.tensor_scalar(
                    out=msk[:], in0=deg_sb[:], scalar1=0.0, scalar2=None, op0=ALU.is_gt
                )
                nc.vector.tensor_tensor(
                    out=dinv_sb[:], in0=sq[:], in1=msk[:], op=ALU.mult
                )

                # replicate -dinv across partitions: drep1[f, t*128+d] = -dinv[d, t]
                pdv = degpp.tile([P, P], dt.float32, tag="pdv", space="PSUM", bufs=1)
                nc.tensor.transpose(out=pdv[:DT, :], in_=dinv_sb[:], identity=ident_f[:])
                dvt = degp.tile([DT, P], dt.float32, tag="dvt")
                nc.vector.tensor_copy(dvt[:], pdv[:DT, :])
                nc.sync.dma_start(
                    out=dinv_d[:].rearrange("(t p) -> t p", p=P), in_=dvt[:]
                )
                r0 = 0
                while r0 < DTP:
                    rb = min(4 * P, DTP - r0)
                    drow = degp.tile([1, 4 * P], dt.float32, tag="drow", bufs=2)
                    nc.sync.dma_start(out=drow[:, :rb], in_=dinv_d[None, r0 : r0 + rb])
                    prr = degpp.tile(
                        [P, 4 * P], dt.float32, tag="prr", space="PSUM", bufs=1
                    )
                    nc.tensor.matmul(
                        out=prr[:, :rb],
                        lhsT=ones_sb[:],
                        rhs=drow[:, :rb],
                        start=True,
                        stop=True,
                    )
                    nc.scalar.activation(
                        drep1[:, r0 : r0 + rb],
                        prr[:, :rb],
                        AF.Copy,
                        scale=-1.0,
                    )
                    r0 += rb

            # ================= phase 2: Aw precompute + layers ===============
            with (
                tc.tile_pool(name="awp", bufs=3) as awp,
                tc.tile_pool(name="ttp", bufs=3) as ttp,
                tc.tile_pool(name="tpp", bufs=2, space="PSUM") as tpp,
                tc.tile_pool(name="gp", bufs=3) as gp,
                tc.tile_pool(name="awl", bufs=2) as awl,
                tc.tile_pool(name="ypp", bufs=4, space="PSUM") as ypp,
                tc.tile_pool(name="wp", bufs=2) as wp,
                tc.tile_pool(name="wpp", bufs=2, space="PSUM") as wpp,
            ):
                # ---- Aw precompute: Aw[slot, d] = ew[slot] * onehot(dstrel) --
                drel_sb = res.tile([P, CT], dt.bfloat16, tag="drel_sb")
                nc.sync.dma_start(out=drel_sb[:], in_=dstrel_d[:, :])
                for t in range(DT):
                    cb = int(chunk_base[t])
                    n_ch = int(ct_t[t])
                    ewt1 = awp.tile([P, CTMAX], dt.float32, tag="ewt1")
                    nc.sync.dma_start(
                        out=ewt1[:, :n_ch],
                        in_=ew1_d[cb * P : (cb + n_ch) * P].rearrange(
                            "(j p) -> p j", p=P
                        ),
                    )
                    for j in range(n_ch):
                        A = awp.tile([P, P], dt.bfloat16, tag="A")
                        nc.vector.tensor_tensor(
                            out=A[:],
                            in0=drel_sb[:, cb + j : cb + j + 1].to_broadcast([P, P]),
                            in1=iota_b[:],
                            op=ALU.is_equal,
                        )
                        Aw = awp.tile([P, P], dt.bfloat16, tag="Aw")
                        nc.scalar.activation(
                            Aw[:], A[:], AF.Copy, scale=ewt1[:, j : j + 1]
                        )
                        nc.sync.dma_start(
                            out=aw_d[:, (cb + j) * P : (cb + j + 1) * P], in_=Aw[:]
                        )

                # ---- helpers ----
                def build_table_tile(src_sb, t, xsrc_dram):
                    """Transpose tile t of src_sb, scale by dinv, write rows."""
                    pt = tpp.tile([P, P], dt.bfloat16, tag="pt", space="PSUM")
                    nc.tensor.transpose(
                        out=pt[:],
                        in_=src_sb[:, t * P : (t + 1) * P],
                        identity=ident_b[:],
                    )
                    xs = ttp.tile([P, P], dt.bfloat16, tag="xs")
                    nc.scalar.activation(
                        xs[:], pt[:], AF.Copy, scale=dinv_sb[:, t : t + 1]
                    )
                    nc.sync.dma_start(out=xsrc_dram[t * P : (t + 1) * P, :], in_=xs[:])

                def ag_quarters(xsrc_dram, xfull_q):
                    for q in range(NQ):
                        nc.gpsimd.collective_compute(
                            "AllGather",
                            mybir.AluOpType.bypass,
                            replica_groups=[core_ids],
                            ins=[xsrc_dram[q * QS : (q + 1) * QS, :]],
                            outs=[xfull_q[q][:, :]],
                        )

                def prop_pass(xfull_q, consume):
                    """Gather + one-hot aggregate all dst tiles; consume(t, py)."""
                    for t in range(DT):
                        cb = int(chunk_base[t])
                        n_ch = int(ct_t[t])
                        xg = gp.tile([P, CTMAX, F], dt.bfloat16, tag="xg")
                        joff = 0
                        for q in range(NQ):
                            n_i = int(NIDX[t, q])
                            n_cq = int(CTQ[t, q])
                            if n_cq == 0:
                                continue
                            g0 = int(gidx_off[t, q])
                            nc.gpsimd.dma_gather(
                                out_ap=xg[:, joff : joff + n_cq, :],
                                in_ap=xfull_q[q][:, :],
                                idxs_ap=gidx_sb[:, g0 : g0 + n_i // 16],
                                num_idxs=n_i,
                                num_idxs_reg=n_i,
                                elem_size=F,
                                queue_num=q,
                            )
                            joff += n_cq
                        awt = awl.tile([P, CTMAX * P], dt.bfloat16, tag="awt")
                        nc.sync.dma_start(
                            out=awt[:, : n_ch * P],
                            in_=aw_d[:, cb * P : (cb + n_ch) * P],
                        )
                        py = ypp.tile([P, P], dt.float32, tag="py", space="PSUM")
                        for j in range(n_ch):
                            nc.tensor.matmul(
                                out=py[:],
                                lhsT=xg[:, j, :],
                                rhs=awt[:, j * P : (j + 1) * P],
                                start=(j == 0),
                                stop=(j == n_ch - 1),
                            )
                        consume(t, py)

                # ---- initial table A ----
                for t in range(DT):
                    build_table_tile(x_temp, t, xsrc_a)
                ag_quarters(xsrc_a, xfull_a)

                for l in range(NGL):
                    # prop1: t1 = -dinv * agg(tableA); build tableB per tile
                    def consume_t1(t, py):
                        sl = slice(t * P, (t + 1) * P)
                        nc.vector.tensor_tensor(
                            out=t1[:, sl], in0=py[:], in1=drep1[:, sl], op=ALU.mult
                        )
                        build_table_tile(t1, t, xsrc_b)

                    prop_pass(xfull_a, consume_t1)
                    ag_quarters(xsrc_b, xfull_b)

                    # prop2: t2 = 2*(-dinv*agg(tableB)) - x_temp
                    def consume_t2(t, py):
                        sl = slice(t * P, (t + 1) * P)
                        nc.vector.tensor_tensor(
                            out=t2[:, sl], in0=py[:], in1=drep1[:, sl], op=ALU.mult
                        )
                        nc.vector.tensor_scalar(
                            out=t2[:, sl],
                            in0=t2[:, sl],
                            scalar1=2.0,
                            scalar2=None,
                            op0=ALU.mult,
                        )
                        nc.vector.tensor_tensor(
                            out=t2[:, sl], in0=t2[:, sl], in1=x_temp[:, sl], op=ALU.subtract
                        )

                    prop_pass(xfull_b, consume_t2)

                    # W phase: blocks of 512 nodes
                    c0 = 0
                    while c0 < DTP:
                        nb = min(4 * P, DTP - c0)
                        po = wpp.tile([P, 4 * P], dt.float32, tag="po", space="PSUM")
                        nc.tensor.matmul(
                            out=po[:, :nb],
                            lhsT=W0_sb[:, l * F : (l + 1) * F],
                            rhs=x_temp[:, c0 : c0 + nb],
                            start=True,
                            stop=False,
                        )
                        nc.tensor.matmul(
                            out=po[:, :nb],
                            lhsT=Wb_sb[:, (l * 2) * F : (l * 2 + 1) * F],
                            rhs=t1[:, c0 : c0 + nb],
                            start=False,
                            stop=False,
                        )
                        nc.tensor.matmul(
                            out=po[:, :nb],
                            lhsT=Wb_sb[:, (l * 2 + 1) * F : (l * 2 + 2) * F],
                            rhs=t2[:, c0 : c0 + nb],
                            start=False,
                            stop=True,
                        )
                        if l == 0:
                            nc.scalar.activation(
                                x_temp[:, c0 : c0 + nb],
                                po[:, :nb],
                                AF.Relu,
                                bias=bT_sb[:, l : l + 1],
                            )
                        elif l < NGL - 1:
                            xn = wp.tile([P, 4 * P], dt.bfloat16, tag="xn")
                            nc.scalar.activation(
                                xn[:, :nb],
                                po[:, :nb],
                                AF.Relu,
                                bias=bT_sb[:, l : l + 1],
                            )
                            nc.vector.tensor_tensor(
                                out=x_temp[:, c0 : c0 + nb],
                                in0=x_temp[:, c0 : c0 + nb],
                                in1=xn[:, :nb],
                                op=ALU.add,
                            )
                        else:
                            xo = wp.tile([P, 4 * P], dt.float32, tag="xo")
                            nc.scalar.activation(
                                xo[:, :nb],
                                po[:, :nb],
                                AF.Relu,
                                bias=bT_sb[:, l : l + 1],
                            )
                            nc.sync.dma_start(
                                out=outT_d[:, c0 : c0 + nb], in_=xo[:, :nb]
                            )
                        c0 += nb

                    if l < NGL - 1:
                        # table A for next layer from updated x_temp
                        for t in range(DT):
                            build_table_tile(x_temp, t, xsrc_a)
                        ag_quarters(xsrc_a, xfull_a)
    if do_compile:
        nc.compile()
    return nc


# ----------------------------------------------------------------------------
# entry point
# ----------------------------------------------------------------------------
_CACHE = {}


def kernel(**inputs):
    n_nodes, fdim = inputs["features"].shape
    n_edges = inputs["edge_index"].shape[1]
    cfg = Cfg(n_nodes, n_edges, 8)
    in_maps, _meta = host_prep(inputs, cfg)

    from concourse.bass_utils import run_bass_kernel_spmd

    key = (cfg.N, cfg.E, cfg.CT, cfg.CT2, tuple(cfg.NIDX.reshape(-1).tolist()))
    if key not in _CACHE:
        _CACHE[key] = build_nc(cfg)
    nc = _CACHE[key]
    res = run_bass_kernel_spmd(nc, in_maps, core_ids=list(range(cfg.NC)))
    return assemble(res.results, cfg)


if __name__ == "__main__":
    pass


# revision 11
# speedup vs baseline: 1.0258x; 1.0258x over previous
"""AELN-GCN (edge-weighted ChebConv K=3, 4 layers) on 8 TRN2 NeuronCores.

v2: gather-descriptor-bound redesign.

Profiling v1 showed the run was bound by SWDGE descriptor generation for
dma_gather on the GpSimd engine (~8.8ns/row, 82.9% busy).  The ucode runs
each dma_gather on one Q7 core-pair selected by queue_num, so v2:
  - compiles with num_swdge_queues=4 and stripes gather calls across the
    4 queues (4 core-pairs emit descriptors concurrently, ~2.7x).
  - buckets the all-gathered table by shard-quarter so each bucket is
    25088 rows (int16-indexable) and issues one gather call per
    (dst tile, bucket) on queue=bucket.
  - precomputes the one-hot aggregation matrices Aw = onehot(dstrel) * ew
    into DRAM once (they are layer-invariant) and streams them per prop,
    removing the per-chunk is_equal/scale from the propagate loop.
  - keeps x_temp/t1/t2/drep resident in bf16 so cheb matmuls read them
    directly.
  - splits each table AllGather into 4 quarter collectives so the next
    prop's gathers start as soon as their bucket arrives.

Math per propagate (unchanged): y = -dinv[dst] * sum(ew * dinv[src] * x[src])
with the dinv[src] folded into the table rows and -dinv[dst] applied at
PSUM evacuation via a replicated drep tile.
"""

import sys

sys.path.insert(0, "/opt/trn_rl_repo")

import numpy as np
import ml_dtypes

P = 128  # partitions
EF = 64  # edge feature dim
HF = 32  # edgenet hidden dim
F = 128  # node feature dim
NGL = 4
NQ = 4  # shard quarters = gather buckets = swdge queues
GCAP = 1024  # HW cap on idxs per dma_gather call

BF16 = ml_dtypes.bfloat16


# ----------------------------------------------------------------------------
# config
# ----------------------------------------------------------------------------
class Cfg:
    def __init__(self, n_nodes, n_edges, n_cores):
        assert n_nodes % n_cores == 0
        self.N = n_nodes
        self.E = n_edges
        self.NC = n_cores
        self.NSH = n_nodes // n_cores  # real nodes per shard
        self.DT = -(-self.NSH // P)  # dst tiles per core
        self.DTP = self.DT * P  # padded shard rows
        assert self.DTP % NQ == 0
        self.QS = self.DTP // NQ  # quarter size (rows per shard-quarter)
        self.BSZ = self.NC * self.QS  # bucket rows (per-quarter table)
        assert self.BSZ <= 32768
        self.TROWS = self.NC * self.DTP
        # set by host_prep (uniform across cores):
        self.CTQ = None  # [DT, NQ] chunks per (tile, bucket)
        self.CT = None  # total chunks (sum of CTQ)
        self.CT2 = None  # chunks per src tile (deg phase)
        self.NIDX = None  # [DT, NQ] static num_idxs per gather call
        self.CTB = 0  # legacy (cache key)

    @property
    def S(self):
        return self.CT * P  # uniform slot count

    @property
    def EP2(self):
        return self.DT * self.CT2 * P


# ----------------------------------------------------------------------------
# host prep
# ----------------------------------------------------------------------------
def _shard_order(node_of_edge, cfg):
    """Per-core edge lists grouped by local tile of `node_of_edge`."""
    cores = []
    max_ct = 1
    order = np.argsort(node_of_edge, kind="stable")
    node_sorted = node_of_edge[order]
    for c in range(cfg.NC):
        lo = np.searchsorted(node_sorted, c * cfg.NSH)
        hi = np.searchsorted(node_sorted, (c + 1) * cfg.NSH)
        eids = order[lo:hi]
        locs = node_sorted[lo:hi] - c * cfg.NSH
        tiles = locs // P
        counts = np.bincount(tiles, minlength=cfg.DT)
        max_ct = max(max_ct, int(-(-counts.max() // P)))
        cores.append((eids, locs, tiles, counts))
    return cores, max_ct


def _fill_slots(cores, cfg, ct):
    """Baseline-style [DT, P, CT] layout (for the deg phase)."""
    out = []
    for eids, locs, tiles, counts in cores:
        eid = np.full((cfg.DT, ct * P), -1, dtype=np.int64)
        rel = np.full((cfg.DT, ct * P), -1, dtype=np.int32)
        starts = np.zeros(cfg.DT + 1, dtype=np.int64)
        np.cumsum(counts, out=starts[1:])
        for t in range(cfg.DT):
            n_t = counts[t]
            if n_t == 0:
                continue
            sl = slice(starts[t], starts[t + 1])
            eid[t, :n_t] = eids[sl]
            rel[t, :n_t] = locs[sl] - t * P
        eid = eid.reshape(cfg.DT, ct, P).transpose(0, 2, 1).copy()
        rel = rel.reshape(cfg.DT, ct, P).transpose(0, 2, 1).copy()
        out.append((eid, rel))
    return out


def host_prep(inputs, cfg):
    feats = np.asarray(inputs["features"], dtype=np.float32)
    egin = np.asarray(inputs["edgenet_input"], dtype=np.float32)
    E1 = np.asarray(inputs["E1"], dtype=np.float32)
    e1b = np.asarray(inputs["e1b"], dtype=np.float32)
    E2 = np.asarray(inputs["E2"], dtype=np.float32)
    e2b = np.asarray(inputs["e2b"], dtype=np.float32)
    W = np.asarray(inputs["W"], dtype=np.float32)
    b = np.asarray(inputs["b"], dtype=np.float32)
    ei = np.asarray(inputs["edge_index"])
    src = ei[0].astype(np.int64)
    dst = ei[1].astype(np.int64)

    # node -> (bucket q, row within bucket)
    shard = src // cfg.NSH
    local = src % cfg.NSH  # < NSH <= DTP
    q_of_src = local // cfg.QS
    brow_of_src = shard * cfg.QS + (local % cfg.QS)

    # ---- dst-shard slot assignment, grouped by (tile, src-bucket) ----
    dcores, _ = _shard_order(dst, cfg)
    percore = []
    cnt_tq = np.zeros((cfg.NC, cfg.DT, NQ), dtype=np.int64)
    for c, (eids, locs, tiles, counts) in enumerate(dcores):
        q = q_of_src[eids]
        key = tiles * NQ + q
        order = np.argsort(key * (cfg.BSZ + 1) + brow_of_src[eids], kind="stable")
        eids_s = eids[order]
        key_s = key[order]
        cnt = np.bincount(key_s, minlength=cfg.DT * NQ).reshape(cfg.DT, NQ)
        cnt_tq[c] = cnt
        percore.append((eids_s, locs[order], key_s))

    # uniform chunk layout from max-over-cores counts
    maxcnt = cnt_tq.max(axis=0)  # [DT, NQ]
    # round num_idxs up to full 128-slot chunks: every slot is written by the
    # gather (pad idxs hit row 0), so no uninitialized SBUF reaches the matmul
    # (0 * garbage-NaN would poison PSUM).
    nidx = -(-np.maximum(maxcnt, 128) // P) * P  # static num_idxs, %128
    ctq = nidx // P  # chunks per (t, q)
    # split oversize calls is not supported; assert under HW cap
    assert nidx.max() <= GCAP, nidx.max()
    cfg.CTQ = ctq
    cfg.NIDX = nidx
    ct_t = ctq.sum(axis=1)  # [DT]
    cfg.CT = int(ct_t.sum())
    chunk_base = np.zeros(cfg.DT, dtype=np.int64)  # first chunk id of tile
    np.cumsum(ct_t[:-1], out=chunk_base[1:])

    # src-shard layout for deg (unchanged from v1)
    scores, ct_s = _shard_order(src, cfg)
    cfg.CT2 = ct_s
    while (cfg.DT * cfg.CT2) % 4 != 0:
        cfg.CT2 += 1
    sslots = _fill_slots(scores, cfg, cfg.CT2)

    eg_ext = np.concatenate([egin, np.zeros((1, EF), np.float32)], axis=0)

    S = cfg.S
    in_maps = []
    for c in range(cfg.NC):
        eids_s, locs_s, key_s = percore[c]
        # slot id per edge: chunks laid out [tile][q-segments]
        eid_slot = np.full(S, -1, dtype=np.int64)  # edge id per slot
        rel_slot = np.full(S, -1, dtype=np.int32)  # dstrel per slot
        gidx_cols = int(nidx.sum() // 16)
        gidx = np.zeros((16, gidx_cols), dtype=np.int16)
        starts = np.zeros(cfg.DT * NQ + 1, dtype=np.int64)
        np.cumsum(cnt_tq[c].reshape(-1), out=starts[1:])
        gcol = 0
        for t in range(cfg.DT):
            sbase = chunk_base[t] * P
            joff = 0
            for q in range(NQ):
                n_c = int(cnt_tq[c, t, q])
                sl = slice(starts[t * NQ + q], starts[t * NQ + q] + n_c)
                es = eids_s[sl]
                s0 = sbase + joff * P
                eid_slot[s0 : s0 + n_c] = es
                rel_slot[s0 : s0 + n_c] = locs_s[sl] - t * P
                # gather idx payload for this call, padded to nidx[t, q]
                n4 = int(nidx[t, q])
                idxs = np.zeros(n4, dtype=np.int16)
                if n_c > 0:
                    idxs[:n_c] = brow_of_src[es].astype(np.int16)
                ncols = n4 // 16
                gidx[:, gcol : gcol + ncols] = idxs.reshape(ncols, 16).T
                gcol += ncols
                joff += int(ctq[t, q])
        assert gcol == gidx_cols
        gidx_t = np.ascontiguousarray(np.tile(gidx, (8, 1)))

        # per-slot streams in chunk-major [chunk, p] order
        eg = eg_ext[np.where(eid_slot >= 0, eid_slot, len(egin))]  # [S, 64]
        dstrel = rel_slot.reshape(cfg.CT, P).T.astype(BF16)  # [P, CT]

        eid_s2, rel_s2 = sslots[c]

        featT = np.zeros((F, cfg.DTP), BF16)
        sh = feats[c * cfg.NSH : (c + 1) * cfg.NSH]
        featT[:, : cfg.NSH] = sh.T.astype(BF16)

        m = {
            "featT": featT,  # [F, DTP] bf16
            "eg": np.ascontiguousarray(eg.T.astype(BF16)),  # [64, S] bf16
            "eg2": np.ascontiguousarray(
                eg_ext[eid_s2.reshape(-1)].T.astype(BF16)
            ),  # [64, EP2]
            "gidx": gidx_t,  # [128, gidx_cols] i16
            "dstrel": dstrel,  # [P, CT] bf16
            "srcrel2": rel_s2.astype(np.float32),  # [DT, P, CT2] f32
            "E1": E1.astype(BF16),
            "e1b": e1b.reshape(HF, 1),
            "E2": E2.astype(BF16),
            "e2b": e2b.reshape(1, 1),
            "W": W,
            "bT": np.ascontiguousarray(b.T),  # [128, NGL]
        }
        in_maps.append(m)
    return in_maps, None


# ----------------------------------------------------------------------------
# assemble final output
# ----------------------------------------------------------------------------
def assemble(results, cfg):
    out = np.zeros((cfg.N, F), np.float32)
    for c in range(cfg.NC):
        outT = np.asarray(results[c]["outT"], dtype=np.float32)
        out[c * cfg.NSH : (c + 1) * cfg.NSH] = outT[:, : cfg.NSH].T
    return out


# ----------------------------------------------------------------------------
# bass kernel builder
# ----------------------------------------------------------------------------
def build_nc(cfg, do_compile=True):
    import concourse.bass as bass
    import concourse.bacc as bacc
    import concourse.mybir as mybir
    import concourse.tile as tile
    from concourse.masks import make_identity

    dt = mybir.dt
    AF = mybir.ActivationFunctionType
    ALU = mybir.AluOpType

    NC, DT, CT, CT2, DTP = cfg.NC, cfg.DT, cfg.CT, cfg.CT2, cfg.DTP
    QS, BSZ = cfg.QS, cfg.BSZ
    CTQ, NIDX = cfg.CTQ, cfg.NIDX
    S, EP2 = cfg.S, cfg.EP2
    core_ids = list(range(NC))
    ct_t = CTQ.sum(axis=1)
    CTMAX = int(ct_t.max())
    chunk_base = np.zeros(DT, dtype=np.int64)
    np.cumsum(ct_t[:-1], out=chunk_base[1:])
    # gather idx column offset per (t, q)
    gidx_off = np.zeros((DT, NQ), dtype=np.int64)
    np.cumsum((NIDX // 16).reshape(-1)[:-1], out=gidx_off.reshape(-1)[1:])
    GIDX_COLS = int((NIDX // 16).sum())
    # last tile index touching each shard quarter (gates the quarter AllGather)
    ag_tile = [min(DT - 1, (QS * (q + 1) - 1) // P) for q in range(NQ)]

    nc = bacc.Bacc(num_swdge_queues=NQ)

    # ---- external I/O ----
    featT_d = nc.declare_dram_parameter("featT", [F, DTP], dt.bfloat16, isOutput=False)
    eg_d = nc.declare_dram_parameter("eg", [EF, S], dt.bfloat16, isOutput=False)
    eg2_d = nc.declare_dram_parameter("eg2", [EF, EP2], dt.bfloat16, isOutput=False)
    gidx_d = nc.declare_dram_parameter("gidx", [P, GIDX_COLS], dt.int16, isOutput=False)
    dstrel_d = nc.declare_dram_parameter("dstrel", [P, CT], dt.bfloat16, isOutput=False)
    srcrel2_d = nc.declare_dram_parameter(
        "srcrel2", [DT, P, CT2], dt.float32, isOutput=False
    )
    E1_d = nc.declare_dram_parameter("E1", [EF, HF], dt.bfloat16, isOutput=False)
    e1b_d = nc.declare_dram_parameter("e1b", [HF, 1], dt.float32, isOutput=False)
    E2_d = nc.declare_dram_parameter("E2", [HF, 1], dt.bfloat16, isOutput=False)
    e2b_d = nc.declare_dram_parameter("e2b", [1, 1], dt.float32, isOutput=False)
    W_d = nc.declare_dram_parameter("W", [NGL, 3, F, F], dt.float32, isOutput=False)
    bT_d = nc.declare_dram_parameter("bT", [F, NGL], dt.float32, isOutput=False)
    outT_d = nc.declare_dram_parameter("outT", [F, DTP], dt.float32, isOutput=True)

    # ---- internal dram ----
    ew1_d = nc.dram_tensor("ew1_buf", [S], dt.float32)
    ew2_d = nc.dram_tensor("ew2_buf", [EP2], dt.float32)
    dinv_d = nc.dram_tensor("dinv_buf", [DTP], dt.float32)
    aw_d = nc.dram_tensor("aw_buf", [P, CT * P], dt.bfloat16)
    xsrc_a = nc.dram_tensor("xsrc_a", [DTP, F], dt.bfloat16)
    xsrc_b = nc.dram_tensor("xsrc_b", [DTP, F], dt.bfloat16)
    xfull_a = [
        nc.dram_tensor(f"xfull_a{q}", [BSZ, F], dt.bfloat16, addr_space="Shared")
        for q in range(NQ)
    ]
    xfull_b = [
        nc.dram_tensor(f"xfull_b{q}", [BSZ, F], dt.bfloat16, addr_space="Shared")
        for q in range(NQ)
    ]

    EB = 512  # edgenet block columns (psum bank limit)

    with tile.TileContext(nc, num_cores=NC) as tc:
        with (
            tc.tile_pool(name="res", bufs=1) as res,
            tc.tile_pool(name="resw", bufs=1) as resw,
        ):
            # ---- resident tiles ----
            x_temp = res.tile([F, DTP], dt.bfloat16, tag="x_temp")
            t1 = res.tile([F, DTP], dt.bfloat16, tag="t1")
            t2 = res.tile([F, DTP], dt.bfloat16, tag="t2")
            drep1 = res.tile([F, DTP], dt.bfloat16, tag="drep1")  # -dinv replicated
            dinv_sb = res.tile([P, DT], dt.float32, tag="dinv_sb")
            deg_sb = res.tile([P, DT], dt.float32, tag="deg_sb")
            ident_f = res.tile([P, P], dt.float32, tag="ident_f")
            ident_b = res.tile([P, P], dt.bfloat16, tag="ident_b")
            iota_i = res.tile([P, P], dt.int32, tag="iota_i")
            iota_b = res.tile([P, P], dt.bfloat16, tag="iota_b")
            iota_f = res.tile([P, P], dt.float32, tag="iota_f")
            ones_sb = res.tile([1, P], dt.float32, tag="ones_sb")
            gidx_sb = res.tile([P, GIDX_COLS], dt.int16, tag="gidx_sb")
            E1_sb = resw.tile([EF, HF], dt.bfloat16, tag="E1_sb")
            e1b_sb = resw.tile([HF, 1], dt.float32, tag="e1b_sb")
            E2_sb = resw.tile([HF, 1], dt.bfloat16, tag="E2_sb")
            e2b_sb = resw.tile([1, 1], dt.float32, tag="e2b_sb")
            bT_sb = resw.tile([F, NGL], dt.float32, tag="bT_sb")
            W0_sb = resw.tile([F, NGL * F], dt.bfloat16, tag="W0_sb")
            Wb_sb = resw.tile([F, NGL * 2 * F], dt.bfloat16, tag="Wb_sb")

            make_identity(nc, ident_f[:])
            nc.vector.tensor_copy(ident_b[:], ident_f[:])
            nc.gpsimd.iota(iota_i[:], pattern=[[1, P]], base=0, channel_multiplier=0)
            nc.vector.tensor_copy(iota_b[:], iota_i[:])
            nc.vector.tensor_copy(iota_f[:], iota_i[:])
            nc.gpsimd.memset(ones_sb[:], 1.0)
            nc.sync.dma_start(out=gidx_sb[:], in_=gidx_d[:, :])
            nc.sync.dma_start(out=E1_sb[:], in_=E1_d[:, :])
            nc.sync.dma_start(out=e1b_sb[:], in_=e1b_d[:, :])
            nc.sync.dma_start(out=E2_sb[:], in_=E2_d[:, :])
            nc.sync.dma_start(out=e2b_sb[:], in_=e2b_d[:, :])
            nc.sync.dma_start(out=bT_sb[:], in_=bT_d[:, :])
            for l in range(NGL):
                nc.gpsimd.dma_start(
                    out=W0_sb[:, l * F : (l + 1) * F], in_=W_d[l, 0, :, :]
                )
                for k in (1, 2):
                    nc.gpsimd.dma_start(
                        out=Wb_sb[:, (l * 2 + k - 1) * F : (l * 2 + k) * F],
                        in_=W_d[l, k, :, :],
                    )
            nc.sync.dma_start(out=x_temp[:], in_=featT_d[:, :])

            # ================= phase 1: edgenet (both orders) + deg =========
            with (
                tc.tile_pool(name="egp", bufs=3) as egp,
                tc.tile_pool(name="hp", bufs=2) as hp,
                tc.tile_pool(name="hpp", bufs=2, space="PSUM") as hpp,
                tc.tile_pool(name="ewp", bufs=2) as ewp,
                tc.tile_pool(name="ewpp", bufs=2, space="PSUM") as ewpp,
                tc.tile_pool(name="degp", bufs=3) as degp,
                tc.tile_pool(name="degpp", bufs=2, space="PSUM") as degpp,
            ):

                def edgenet(eg_dram, ew_dram, n_cols):
                    c0 = 0
                    while c0 < n_cols:
                        nb = min(EB, n_cols - c0)
                        egt = egp.tile([EF, EB], dt.bfloat16, tag="egt")
                        nc.sync.dma_start(
                            out=egt[:, :nb], in_=eg_dram[:, c0 : c0 + nb]
                        )
                        ph = hpp.tile([HF, EB], dt.float32, tag="ph", space="PSUM")
                        nc.tensor.matmul(
                            out=ph[:, :nb], lhsT=E1_sb[:], rhs=egt[:, :nb], start=True, stop=True
                        )
                        hs = hp.tile([HF, EB], dt.bfloat16, tag="hs")
                        nc.vector.tensor_tensor(
                            out=hs[:, :nb],
                            in0=ph[:, :nb],
                            in1=e1b_sb[:, :1].to_broadcast([HF, nb]),
                            op=mybir.AluOpType.add,
                        )
                        nc.vector.tensor_scalar_max(hs[:, :nb], hs[:, :nb], 0.0)
                        pw = ewpp.tile([1, EB], dt.float32, tag="pw", space="PSUM")
                        nc.tensor.matmul(
                            out=pw[:, :nb], lhsT=E2_sb[:], rhs=hs[:, :nb], start=True, stop=True
                        )
                        ews = ewp.tile([1, EB], dt.float32, tag="ews")
                        nc.scalar.activation(
                            ews[:, :nb], pw[:, :nb], AF.Sigmoid, bias=e2b_sb[:1, :1]
                        )
                        nc.sync.dma_start(
                            out=ew_dram[None, c0 : c0 + nb],
                            in_=ews[:1, :nb],
                        )
                        c0 += nb

                edgenet(eg2_d, ew2_d, EP2)

                # deg: per src tile, one-hot matmul accumulate
                for t in range(DT):
                    relt = degp.tile([P, CT2], dt.float32, tag="relt")
                    nc.sync.dma_start(out=relt[:], in_=srcrel2_d[t, :, :])
                    ewt = degp.tile([P, CT2], dt.float32, tag="ewt2")
                    nc.sync.dma_start(
                        out=ewt[:],
                        in_=ew2_d[t * P * CT2 : (t + 1) * P * CT2].rearrange(
                            "(p j) -> p j", p=P
                        ),
                    )
                    pd = degpp.tile([P, 1], dt.float32, tag="pd", space="PSUM", bufs=1)
                    for j in range(CT2):
                        A2 = degp.tile([P, P], dt.float32, tag="A2")
                        nc.vector.tensor_tensor(
                            out=A2[:],
                            in0=relt[:, j : j + 1].to_broadcast([P, P]),
                            in1=iota_f[:],
                            op=ALU.is_equal,
                        )
                        nc.tensor.matmul(
                            out=pd[:],
                            lhsT=A2[:],
                            rhs=ewt[:, j : j + 1],
                            start=(j == 0),
                            stop=(j == CT2 - 1),
                        )
                    nc.vector.tensor_copy(deg_sb[:, t : t + 1], pd[:])

                # dinv = (deg>0) / sqrt(max(deg,1e-12))
                mx = degp.tile([P, DT], dt.float32, tag="mx")
                nc.vector.tensor_scalar_max(mx[:], deg_sb[:], 1e-12)
                rc = degp.tile([P, DT], dt.float32, tag="rc")
                nc.vector.reciprocal(rc[:], mx[:])
                sq = degp.tile([P, DT], dt.float32, tag="sq")
                nc.scalar.activation(sq[:], rc[:], AF.Sqrt)
                msk = degp.tile([P, DT], dt.float32, tag="msk")
                nc.vector.tensor_scalar(
                    out=msk[:], in0=deg_sb[:], scalar1=0.0, scalar2=None, op0=ALU.is_gt
                )
                nc.vector.tensor_tensor(
                    out=dinv_sb[:], in0=sq[:], in1=msk[:], op=ALU.mult
                )

                # replicate -dinv across partitions: drep1[f, t*128+d] = -dinv[d, t]
                pdv = degpp.tile([P, P], dt.float32, tag="pdv", space="PSUM", bufs=1)
                nc.tensor.transpose(out=pdv[:DT, :], in_=dinv_sb[:], identity=ident_f[:])
                dvt = degp.tile([DT, P], dt.float32, tag="dvt")
                nc.vector.tensor_copy(dvt[:], pdv[:DT, :])
                nc.sync.dma_start(
                    out=dinv_d[:].rearrange("(t p) -> t p", p=P), in_=dvt[:]
                )
                r0 = 0
                while r0 < DTP:
                    rb = min(4 * P, DTP - r0)
                    drow = degp.tile([1, 4 * P], dt.float32, tag="drow", bufs=2)
                    nc.sync.dma_start(out=drow[:, :rb], in_=dinv_d[None, r0 : r0 + rb])
                    prr = degpp.tile(
                        [P, 4 * P], dt.float32, tag="prr", space="PSUM", bufs=1
                    )
                    nc.tensor.matmul(
                        out=prr[:, :rb],
                        lhsT=ones_sb[:],
                        rhs=drow[:, :rb],
                        start=True,
                        stop=True,
                    )
                    nc.scalar.activation(
                        drep1[:, r0 : r0 + rb],
                        prr[:, :rb],
                        AF.Copy,
                        scale=-1.0,
                    )
                    r0 += rb

                edgenet(eg_d, ew1_d, S)

            # ================= phase 2: Aw precompute + layers ===============
            with (
                tc.tile_pool(name="awp", bufs=3) as awp,
                tc.tile_pool(name="ttp", bufs=3) as ttp,
                tc.tile_pool(name="tpp", bufs=2, space="PSUM") as tpp,
                tc.tile_pool(name="gp", bufs=3) as gp,
                tc.tile_pool(name="awl", bufs=2) as awl,
                tc.tile_pool(name="ypp", bufs=4, space="PSUM") as ypp,
                tc.tile_pool(name="wp", bufs=2) as wp,
                tc.tile_pool(name="wpp", bufs=2, space="PSUM") as wpp,
            ):
                # ---- Aw precompute: Aw[slot, d] = ew[slot] * onehot(dstrel) --
                drel_sb = res.tile([P, CT], dt.bfloat16, tag="drel_sb")
                nc.sync.dma_start(out=drel_sb[:], in_=dstrel_d[:, :])
                for t in range(DT):
                    cb = int(chunk_base[t])
                    n_ch = int(ct_t[t])
                    ewt1 = awp.tile([P, CTMAX], dt.float32, tag="ewt1")
                    nc.sync.dma_start(
                        out=ewt1[:, :n_ch],
                        in_=ew1_d[cb * P : (cb + n_ch) * P].rearrange(
                            "(j p) -> p j", p=P
                        ),
                    )
                    for j in range(n_ch):
                        A = awp.tile([P, P], dt.bfloat16, tag="A")
                        nc.vector.tensor_tensor(
                            out=A[:],
                            in0=drel_sb[:, cb + j : cb + j + 1].to_broadcast([P, P]),
                            in1=iota_b[:],
                            op=ALU.is_equal,
                        )
                        Aw = awp.tile([P, P], dt.bfloat16, tag="Aw")
                        nc.scalar.activation(
                            Aw[:], A[:], AF.Copy, scale=ewt1[:, j : j + 1]
                        )
                        nc.sync.dma_start(
                            out=aw_d[:, (cb + j) * P : (cb + j + 1) * P], in_=Aw[:]
                        )

                # ---- helpers ----
                def build_table_tile(src_sb, t, xsrc_dram):
                    """Transpose tile t of src_sb, scale by dinv, write rows."""
                    pt = tpp.tile([P, P], dt.bfloat16, tag="pt", space="PSUM")
                    nc.tensor.transpose(
                        out=pt[:],
                        in_=src_sb[:, t * P : (t + 1) * P],
                        identity=ident_b[:],
                    )
                    xs = ttp.tile([P, P], dt.bfloat16, tag="xs")
                    nc.scalar.activation(
                        xs[:], pt[:], AF.Copy, scale=dinv_sb[:, t : t + 1]
                    )
                    nc.sync.dma_start(out=xsrc_dram[t * P : (t + 1) * P, :], in_=xs[:])

                def ag_quarters(xsrc_dram, xfull_q):
                    for q in range(NQ):
                        nc.gpsimd.collective_compute(
                            "AllGather",
                            mybir.AluOpType.bypass,
                            replica_groups=[core_ids],
                            ins=[xsrc_dram[q * QS : (q + 1) * QS, :]],
                            outs=[xfull_q[q][:, :]],
                        )

                def prop_pass(xfull_q, consume):
                    """Gather + one-hot aggregate all dst tiles; consume(t, py)."""
                    for t in range(DT):
                        cb = int(chunk_base[t])
                        n_ch = int(ct_t[t])
                        xg = gp.tile([P, CTMAX, F], dt.bfloat16, tag="xg")
                        joff = 0
                        for q in range(NQ):
                            n_i = int(NIDX[t, q])
                            n_cq = int(CTQ[t, q])
                            if n_cq == 0:
                                continue
                            g0 = int(gidx_off[t, q])
                            nc.gpsimd.dma_gather(
                                out_ap=xg[:, joff : joff + n_cq, :],
                                in_ap=xfull_q[q][:, :],
                                idxs_ap=gidx_sb[:, g0 : g0 + n_i // 16],
                                num_idxs=n_i,
                                num_idxs_reg=n_i,
                                elem_size=F,
                                queue_num=q,
                            )
                            joff += n_cq
                        awt = awl.tile([P, CTMAX * P], dt.bfloat16, tag="awt")
                        nc.sync.dma_start(
                            out=awt[:, : n_ch * P],
                            in_=aw_d[:, cb * P : (cb + n_ch) * P],
                        )
                        py = ypp.tile([P, P], dt.float32, tag="py", space="PSUM")
                        for j in range(n_ch):
                            nc.tensor.matmul(
                                out=py[:],
                                lhsT=xg[:, j, :],
                                rhs=awt[:, j * P : (j + 1) * P],
                                start=(j == 0),
                                stop=(j == n_ch - 1),
                            )
                        consume(t, py)

                # ---- initial table A ----
                for t in range(DT):
                    build_table_tile(x_temp, t, xsrc_a)
                ag_quarters(xsrc_a, xfull_a)

                for l in range(NGL):
                    # prop1: t1 = -dinv * agg(tableA); build tableB per tile
                    def consume_t1(t, py):
                        sl = slice(t * P, (t + 1) * P)
                        nc.vector.tensor_tensor(
                            out=t1[:, sl], in0=py[:], in1=drep1[:, sl], op=ALU.mult
                        )
                        build_table_tile(t1, t, xsrc_b)

                    prop_pass(xfull_a, consume_t1)
                    ag_quarters(xsrc_b, xfull_b)

                    # prop2: t2 = 2*(-dinv*agg(tableB)) - x_temp
                    def consume_t2(t, py):
                        sl = slice(t * P, (t + 1) * P)
                        nc.vector.tensor_tensor(
                            out=t2[:, sl], in0=py[:], in1=drep1[:, sl], op=ALU.mult
                        )
                        nc.vector.tensor_scalar(
                            out=t2[:, sl],
                            in0=t2[:, sl],
                            scalar1=2.0,
                            scalar2=None,
                            op0=ALU.mult,
                        )
                        nc.vector.tensor_tensor(
                            out=t2[:, sl], in0=t2[:, sl], in1=x_temp[:, sl], op=ALU.subtract
                        )

                    prop_pass(xfull_b, consume_t2)

                    # W phase: blocks of 512 nodes
                    c0 = 0
                    while c0 < DTP:
                        nb = min(4 * P, DTP - c0)
                        po = wpp.tile([P, 4 * P], dt.float32, tag="po", space="PSUM")
                        nc.tensor.matmul(
                            out=po[:, :nb],
                            lhsT=W0_sb[:, l * F : (l + 1) * F],
                            rhs=x_temp[:, c0 : c0 + nb],
                            start=True,
                            stop=False,
                        )
                        nc.tensor.matmul(
                            out=po[:, :nb],
                            lhsT=Wb_sb[:, (l * 2) * F : (l * 2 + 1) * F],
                            rhs=t1[:, c0 : c0 + nb],
                            start=False,
                            stop=False,
                        )
                        nc.tensor.matmul(
                            out=po[:, :nb],
                            lhsT=Wb_sb[:, (l * 2 + 1) * F : (l * 2 + 2) * F],
                            rhs=t2[:, c0 : c0 + nb],
                            start=False,
                            stop=True,
                        )
                        if l == 0:
                            nc.scalar.activation(
                                x_temp[:, c0 : c0 + nb],
                                po[:, :nb],
                                AF.Relu,
                                bias=bT_sb[:, l : l + 1],
                            )
                        elif l < NGL - 1:
                            xn = wp.tile([P, 4 * P], dt.bfloat16, tag="xn")
                            nc.scalar.activation(
                                xn[:, :nb],
                                po[:, :nb],
                                AF.Relu,
                                bias=bT_sb[:, l : l + 1],
                            )
                            nc.vector.tensor_tensor(
                                out=x_temp[:, c0 : c0 + nb],
                                in0=x_temp[:, c0 : c0 + nb],
                                in1=xn[:, :nb],
                                op=ALU.add,
                            )
                        else:
                            xo = wp.tile([P, 4 * P], dt.float32, tag="xo")
                            nc.scalar.activation(
                                xo[:, :nb],
                                po[:, :nb],
                                AF.Relu,
                                bias=bT_sb[:, l : l + 1],
                            )
                            nc.sync.dma_start(
                                out=outT_d[:, c0 : c0 + nb], in_=xo[:, :nb]
                            )
                        c0 += nb

                    if l < NGL - 1:
                        # table A for next layer from updated x_temp
                        for t in range(DT):
                            build_table_tile(x_temp, t, xsrc_a)
                        ag_quarters(xsrc_a, xfull_a)
    if do_compile:
        nc.compile()
    return nc


# ----------------------------------------------------------------------------
# entry point
# ----------------------------------------------------------------------------
_CACHE = {}


def kernel(**inputs):
    n_nodes, fdim = inputs["features"].shape
    n_edges = inputs["edge_index"].shape[1]
    cfg = Cfg(n_nodes, n_edges, 8)
    in_maps, _meta = host_prep(inputs, cfg)

    from concourse.bass_utils import run_bass_kernel_spmd

    key = (cfg.N, cfg.E, cfg.CT, cfg.CT2, tuple(cfg.NIDX.reshape(-1).tolist()))
    if key not in _CACHE:
        _CACHE[key] = build_nc(cfg)
    nc = _CACHE[key]
    res = run_bass_kernel_spmd(nc, in_maps, core_ids=list(range(cfg.NC)))
    return assemble(res.results, cfg)


if __name__ == "__main__":
    pass
